# revision 17
# baseline (speedup 1.0000x reference)
"""Trainium2 Bass kernel for nn_Actor LSTM scan problem.

Reference computation (per problem):
    xW = x @ W_ih.T + b_ih                       (B, 4H)
    for t in range(D):
        gates = xW + h @ W_hh.T + b_hh           (B, 4H)  order i,f,g,o
        c = sig(f)*c + sig(i)*tanh(g)
        h = sig(o)*tanh(c)
        out[t] = h @ W_out.T + b_out             (B, N)

Shapes: B=512, IN=H=1024, D=64, N=512.

Strategy: pure data parallel over 8 NeuronCores (64 batch rows each, weights
replicated; no collectives). Per core the recurrence runs on the TensorEngine
with the batch as the matmul output-partition dim (M=64), stationary = h.T
K-tiles, moving = W_hh.T. The two free column-group halves of the 128x128 PE
array are used as two concurrent matmul streams (tile_position col split):
group A (psum partitions 0-63) computes gate columns 0-511 of each gate,
group B (psum partitions 64-127) computes gate columns 512-1023. All compute
in bf16 with fp32 PSUM accumulation; c state kept in fp32.

PSUM bank layout per step (bank order): g, i, f, o gates; each bank holds
[128, 512] = A-half rows 0-63 (lo gate cols) / B-half rows 64-127 (hi cols).
xW (+ biases) is pre-computed once on-device and re-injected into PSUM every
step via a single identity-stationary matmul per bank (start=True).
"""

import os
import sys

for _p in ("/opt/trn_rl_repo", "/root/.axon_site/_ro/trn_rl_repo"):
    if os.path.isdir(_p) and _p not in sys.path:
        sys.path.insert(0, _p)

import numpy as np
import ml_dtypes

BF = ml_dtypes.bfloat16

N_CORES = 8
B_LOC = 64    # batch rows per core
H = 1024
IN = 1024
NOUT = 512
KT = 8        # K tiles of 128 along the hidden/contraction dim

# gate blocks in torch order within the 4H dim: i, f, g, o
_GI, _GF, _GG, _GO = 0, 1024, 2048, 3072
# bank order: g, i, f, o ; A = cols 0:512 of each gate, B = cols 512:1024
COLS_A = np.concatenate([
    np.arange(_GG, _GG + 512),
    np.arange(_GI, _GI + 512),
    np.arange(_GF, _GF + 512),
    np.arange(_GO, _GO + 512),
])
COLS_B = COLS_A + 512


def _tile_k(m):
    """(KT*128, C) -> (128, KT, C) so dma is a straight copy into sbuf."""
    c = m.shape[1]
    return np.ascontiguousarray(
        m.reshape(KT, 128, c).transpose(1, 0, 2))


def _prep_shared(W_ih, W_hh, b_ih, b_hh, W_out, b_out):
    """Host-side layout prep of the replicated weights (numpy, float32 in)."""
    WihT = np.asarray(W_ih, np.float32).T   # (IN, 4H)
    WhhT = np.asarray(W_hh, np.float32).T   # (H, 4H)
    WoT = np.asarray(W_out, np.float32).T   # (H, N)
    b4 = (np.asarray(b_ih, np.float32) + np.asarray(b_hh, np.float32))

    d = {}
    d["wiA"] = _tile_k(WihT[:, COLS_A]).astype(BF)
    d["wiB"] = _tile_k(WihT[:, COLS_B]).astype(BF)
    d["whA"] = _tile_k(WhhT[:, COLS_A]).astype(BF)
    d["whB"] = _tile_k(WhhT[:, COLS_B]).astype(BF)
    d["woA"] = _tile_k(WoT[:, :256]).astype(BF)
    d["woB"] = _tile_k(WoT[:, 256:]).astype(BF)
    d["b42"] = np.stack([b4[COLS_A], b4[COLS_B]]).astype(BF)
    bo = np.asarray(b_out, np.float32)
    d["bo2"] = np.stack([bo[:256], bo[256:]]).astype(BF)
    sel2 = np.zeros((2, 128), np.float32)
    sel2[0, :64] = 1.0
    sel2[1, 64:] = 1.0
    d["sel2"] = sel2.astype(BF)
    idm = np.zeros((128, 128), np.float32)
    idm[:128, :128] = np.eye(128)
    d["idm"] = idm.astype(BF)
    return d


def _prep_core(x_sh, h0_sh, c0_sh):
    """Per-core input layout prep."""
    d = {}
    d["xaT"] = _tile_k(np.asarray(x_sh, np.float32).T).astype(BF)
    d["h0T"] = _tile_k(np.asarray(h0_sh, np.float32).T).astype(BF)
    c0 = np.asarray(c0_sh, np.float32)
    d["c0s"] = np.concatenate([c0[:, :512], c0[:, 512:]], axis=0)
    return d


def build_kernel(D):
    import concourse.bass as bass
    import concourse.tile as tile
    from concourse import bacc, mybir
    from contextlib import ExitStack

    f32 = mybir.dt.float32
    bf16 = mybir.dt.bfloat16
    Sig = mybir.ActivationFunctionType.Sigmoid
    Tanh = mybir.ActivationFunctionType.Tanh

    nc = bacc.Bacc()

    def din(name, shape, dt):
        return nc.declare_dram_parameter(name, list(shape), dt, isOutput=False)

    xaT_d = din("xaT", (128, KT, 64), bf16)
    h0T_d = din("h0T", (128, KT, 64), bf16)
    c0s_d = din("c0s", (128, 512), f32)
    wiA_d = din("wiA", (128, KT, 2048), bf16)
    wiB_d = din("wiB", (128, KT, 2048), bf16)
    whA_d = din("whA", (128, KT, 2048), bf16)
    whB_d = din("whB", (128, KT, 2048), bf16)
    woA_d = din("woA", (128, KT, 256), bf16)
    woB_d = din("woB", (128, KT, 256), bf16)
    b42_d = din("b42", (2, 2048), bf16)
    bo2_d = din("bo2", (2, 256), bf16)
    sel2_d = din("sel2", (2, 128), bf16)
    idm_d = din("idm", (128, 128), bf16)
    out_d = nc.declare_dram_parameter("out", [D, B_LOC, NOUT], f32, isOutput=True)

    with tile.TileContext(nc) as tc, ExitStack() as ctx:
        wp = ctx.enter_context(tc.tile_pool(name="wp", bufs=1))
        sp = ctx.enter_context(tc.tile_pool(name="sp", bufs=2))
        op = ctx.enter_context(tc.tile_pool(name="op", bufs=3))
        pp = ctx.enter_context(
            tc.tile_pool(name="pp", bufs=1, space=bass.MemorySpace.PSUM))

        # ---- constants / weights into SBUF ----
        idm = wp.tile([128, 128], bf16, tag="idm")
        nc.sync.dma_start(idm[:], idm_d[:])
        sel2 = wp.tile([2, 128], bf16, tag="sel2")
        nc.sync.dma_start(sel2[:], sel2_d[:])

        xa = wp.tile([128, KT, 64], bf16, tag="xa")
        nc.sync.dma_start(xa[:], xaT_d[:])
        wi = {}
        for g, dd in (("A", wiA_d), ("B", wiB_d)):
            wi[g] = wp.tile([128, KT, 2048], bf16, tag=f"wi{g}", name=f"wi{g}")
            nc.sync.dma_start(wi[g][:], dd[:])
        b42 = wp.tile([2, 2048], bf16, tag="b42")
        nc.sync.dma_start(b42[:], b42_d[:])
        wh = {}
        for g, dd in (("A", whA_d), ("B", whB_d)):
            wh[g] = wp.tile([128, KT, 2048], bf16, tag=f"wh{g}", name=f"wh{g}")
            nc.sync.dma_start(wh[g][:], dd[:])
        wo = {}
        for g, dd in (("A", woA_d), ("B", woB_d)):
            wo[g] = wp.tile([128, KT, 256], bf16, tag=f"wo{g}", name=f"wo{g}")
            nc.sync.dma_start(wo[g][:], dd[:])
        bo2 = wp.tile([2, 256], bf16, tag="bo2")
        nc.sync.dma_start(bo2[:], bo2_d[:])

        # initial state: hT k-tiles 0-3 in hta, 4-7 in htb
        hta = sp.tile([128, 4, 64], bf16, tag="hta", bufs=2, name="hta_init")
        nc.sync.dma_start(hta[:], h0T_d[:, 0:4, :])
        htb = sp.tile([128, 4, 64], bf16, tag="htb", bufs=2, name="htb_init")
        nc.sync.dma_start(htb[:], h0T_d[:, 4:8, :])
        c_cur = sp.tile([128, 512], f32, tag="c", bufs=2)
        nc.sync.dma_start(c_cur[:], c0s_d[:])

        PO = {"A": 0, "B": 64}  # partition offset per group

        def ht_sl(ha, hb, k):
            return ha[:, k, :] if k < 4 else hb[:, k - 4, :]

        # ---- xW precompute: xw[p, 2048] bf16 (A half rows 0-63, B rows 64-127)
        xw = wp.tile([128, 2048], bf16, tag="xw")
        for b in range(4):
            sl = slice(512 * b, 512 * b + 512)
            pt = pp.tile([128, 512], f32, tag=f"pb{b}",
                         bufs=2 if b == 3 else 1,
                         name=f"xwpre{b}")
            nc.tensor.matmul(pt[:, :], sel2[:, :], b42[:, sl],
                             start=True, stop=False,
                             skip_group_check=True)
            for k in range(KT):
                for g in ("A", "B"):
                    po = PO[g]
                    hsl = slice(po, po + 64)
                    nc.tensor.matmul(pt[hsl, :], xa[:, k, :],
                                     wi[g][:, k, sl],
                                     start=False,
                                     stop=(k == KT - 1),
                                     skip_group_check=True)
            nc.vector.tensor_copy(xw[:, sl], pt[:, :])

        # ---- helpers ----
        def emit_outproj(t, ha, hb):
            pout = pp.tile([128, 256], f32, tag="pout", bufs=1,
                           name=f"pout{t}")
            nc.tensor.matmul(pout[:, :], sel2[:, :], bo2[:, :],
                             start=True, stop=False,
                             skip_group_check=True)
            for k in range(KT):
                for g in ("A", "B"):
                    po = PO[g]
                    hsl = slice(po, po + 64)
                    nc.tensor.matmul(pout[hsl, :], ht_sl(ha, hb, k),
                                     wo[g][:, k, :],
                                     start=False, stop=(k == KT - 1),
                                     skip_group_check=True)
            ost = op.tile([128, 256], f32, tag="ost", bufs=4,
                          name=f"ost{t}")
            nc.vector.tensor_copy(ost[:, :], pout[:, :])
            dst = out_d[t].rearrange("b (c n) -> c b n", c=2)
            nc.sync.dma_start(dst, ost[:, :])

        # ---- the scan ----
        for t in range(D):
            # gates matmuls (hta/htb hold hT(h_{t-1}))
            pts = []
            for b in range(4):
                sl = slice(512 * b, 512 * b + 512)
                # pb3 (o gate) is double-buffered: its ACT read (sig o) is the
                # last epilogue op, and bufs=2 lets next step's xW inject run
                # without waiting on it.
                pt = pp.tile([128, 512], f32, tag=f"pb{b}",
                             bufs=2 if b == 3 else 1,
                             name=f"pt{t}_{b}")
                if b == 3:
                    # o bank runs as two independent 256-col accumulation
                    # groups: sig(o) of the first half fires while the second
                    # half's matmuls still stream, pulling h forward.
                    for oh in range(2):
                        osl = slice(256 * oh, 256 * oh + 256)
                        wsl = slice(512 * b + 256 * oh, 512 * b + 256 * oh + 256)
                        nc.tensor.matmul(pt[:, osl], idm[:, :], xw[:, wsl],
                                         start=True, stop=False,
                                         skip_group_check=True)
                        for k in range(KT):
                            for g in ("A", "B"):
                                po = PO[g]
                                hsl = slice(po, po + 64)
                                nc.tensor.matmul(pt[hsl, osl],
                                                 ht_sl(hta, htb, k),
                                                 wh[g][:, k, wsl],
                                                 start=False,
                                                 stop=(k == KT - 1),
                                                 skip_group_check=True)
                else:
                    # inject xW (+biases) into the whole bank in one
                    # full-array MM
                    nc.tensor.matmul(pt[:, :], idm[:, :], xw[:, sl],
                                     start=True, stop=False,
                                     skip_group_check=True)
                    for k in range(KT):
                        for g in ("A", "B"):
                            po = PO[g]
                            hsl = slice(po, po + 64)
                            nc.tensor.matmul(pt[hsl, :], ht_sl(hta, htb, k),
                                             wh[g][:, k, sl],
                                             start=False, stop=(k == KT - 1),
                                             skip_group_check=True)
                pts.append(pt)

            # fill TE while epilogue runs: output projection of previous step
            if t > 0:
                emit_outproj(t - 1, hta, htb)

            # epilogue: c = sig(f)*c + sig(i)*tanh(g); h = sig(o)*tanh(c)
            # banks: 0=g, 1=i, 2=f, 3=o.  f/o + the c->h chain are split in
            # 256-col halves so the tail latency pipelines; so/tcc are bf16
            # to get 2x DVE mode on the final h multiply.
            HL = [slice(0, 256), slice(256, 512)]
            tg = op.tile([128, 512], f32, tag="tg", bufs=2, name=f"tg{t}")
            nc.scalar.activation(tg[:, :], pts[0][:, :], Tanh)
            si = op.tile([128, 512], f32, tag="si", bufs=2, name=f"si{t}")
            nc.scalar.activation(si[:, :], pts[1][:, :], Sig)
            sf = op.tile([128, 512], f32, tag="sf", bufs=2, name=f"sf{t}")
            nc.scalar.activation(sf[:, 0:256], pts[2][:, 0:256], Sig)
            nc.scalar.activation(sf[:, 256:512], pts[2][:, 256:512], Sig)

            t2 = op.tile([128, 512], f32, tag="t2", bufs=2, name=f"t2_{t}")
            nc.vector.tensor_mul(t2[:, :], si[:, :], tg[:, :])

            t1 = op.tile([128, 512], f32, tag="t1", bufs=2, name=f"t1_{t}")
            c_new = sp.tile([128, 512], f32, tag="c", bufs=2, name=f"c{t}")
            tcc = op.tile([128, 512], bf16, tag="tcc", bufs=2, name=f"tcc{t}")
            so = op.tile([128, 512], bf16, tag="so", bufs=2, name=f"so{t}")
            hh = op.tile([128, 512], bf16, tag="hh", bufs=2, name=f"hh{t}")
            for h_ in range(2):
                hsl = HL[h_]
                nc.vector.tensor_mul(t1[:, hsl], sf[:, hsl], c_cur[:, hsl])
                nc.vector.tensor_add(c_new[:, hsl], t1[:, hsl], t2[:, hsl])
            # ACT tail in input-ready order: tcc1, so1 (at o-half0 stop),
            # tcc2, so2; hh halves land right at the o bank boundary.
            nc.scalar.activation(tcc[:, 0:256], c_new[:, 0:256], Tanh)
            nc.scalar.activation(so[:, 0:256], pts[3][:, 0:256], Sig)
            nc.scalar.activation(tcc[:, 256:512], c_new[:, 256:512], Tanh)
            nc.scalar.activation(so[:, 256:512], pts[3][:, 256:512], Sig)
            for h_ in range(2):
                hsl = HL[h_]
                nc.vector.tensor_mul(hh[:, hsl], so[:, hsl], tcc[:, hsl])
            c_cur = c_new

            # transpose h -> hT tiles: regular matmuls with identity moving
            # operand; the upper batch half reads partitions 64-127 directly
            # (row tile position 64), no DRAM roundtrip.
            pa = pp.tile([128, 256], f32, tag="pa", bufs=1, name=f"pa{t}")
            pb = pp.tile([128, 256], f32, tag="pb_t", bufs=1, name=f"pbt{t}")
            for j in range(4):
                nc.tensor.matmul(pa[:, 64 * j:64 * j + 64],
                                 hh[0:64, 128 * j:128 * j + 128],
                                 idm[0:64, 0:64],
                                 start=True, stop=True,
                                 skip_group_check=True)
            for j in range(4):
                nc.tensor.matmul(pb[:, 64 * j:64 * j + 64],
                                 hh[64:128, 128 * j:128 * j + 128],
                                 idm[64:128, 64:128],
                                 start=True, stop=True,
                                 skip_group_check=True)
            hta = sp.tile([128, 4, 64], bf16, tag="hta", bufs=2,
                          name=f"hta{t}")
            htb = sp.tile([128, 4, 64], bf16, tag="htb", bufs=2,
                          name=f"htb{t}")
            nc.vector.tensor_copy(
                hta[:, 0:2, :], pa[:, 0:128].rearrange("p (k n) -> p k n", k=2))
            nc.vector.tensor_copy(
                hta[:, 2:4, :], pa[:, 128:256].rearrange("p (k n) -> p k n", k=2))
            nc.vector.tensor_copy(
                htb[:, 0:2, :], pb[:, 0:128].rearrange("p (k n) -> p k n", k=2))
            nc.vector.tensor_copy(
                htb[:, 2:4, :], pb[:, 128:256].rearrange("p (k n) -> p k n", k=2))

        emit_outproj(D - 1, hta, htb)

    if not nc.is_finalized():
        nc.finalize()
    return nc


def _fix_matmul_waits(nc, mybir):
    """Walrus allows only one sync-wait slot per compute instruction
    (PE matmul / ldweights, DVE, ACT, DMA trigger). Tile sometimes
    emits 2-3. Legalize each engine stream:
      1. drop waits on the instruction's own update semaphore (engine
         streams and DMA rings complete in order - guaranteed by
         program order);
      2. hoist surplus waits onto the nearest preceding same-engine
         instruction with a free wait slot (conservative: a wait moved
         earlier on the same serial stream still gates the original
         instruction; the producers of such waits are always at least
         one pool-buf generation older, so no cycles arise within the
         short hoist window).
    """
    LIMIT1 = {"InstMatmult", "InstLdweights", "InstTensorTensor",
              "InstTensorCopy", "InstActivation", "InstTensorScalarPtr",
              "InstMemset", "InstTensorReduce", "InstCopyPredicated",
              "InstReciprocal", "InstDMACopy"}
    for fn in nc.m.functions:
        for blk in fn.blocks:
            insts = [i for i in blk.instructions if type(i).__name__
                     in LIMIT1]
            by_eng = {}
            for inst in insts:
                by_eng.setdefault(str(inst.engine), []).append(inst)
            for eng, stream in by_eng.items():
                for pos, inst in enumerate(stream):
                    si = inst.sync_info
                    if si is None or len(si.on_wait) <= 1:
                        continue
                    own = {u.ant_name for u in si.on_update}
                    if type(inst).__name__ == "InstDMACopy" and \
                            os.environ.get("KFIX_DMA_KEEP_RING", "0") == "1":
                        waits = list(si.on_wait)
                    else:
                        waits = [w for w in si.on_wait
                                 if w.ant_name not in own]
                    if len(waits) > 1:
                        # keep a cross-engine compute (RAW) wait on the
                        # instruction; hoist the DMA (pool-reuse WAR)
                        # waits onto nearby predecessors
                        if type(inst).__name__ == "InstDMACopy":
                            keep = next(
                                (w for w in waits if w.ant_name in own),
                                waits[-1])
                        else:
                            keep = next(
                                (w for w in waits
                                 if "DMA" not in w.ant_name),
                                waits[-1])
                        surplus = [w for w in waits if w is not keep]
                        waits = [keep]
                        j = pos - 1
                        while surplus and j >= 0 and j >= pos - 24:
                            cand = stream[j]
                            csi = cand.sync_info
                            cw = list(csi.on_wait) if csi else []
                            if len(cw) == 0:
                                w = surplus.pop()
                                cand.sync_info = mybir.SyncInfo(
                                    on_wait=[w],
                                    on_update=list(csi.on_update)
                                    if csi else [])
                            elif len(cw) == 1:
                                # merge if same semaphore
                                for w in list(surplus):
                                    if w.ant_name == cw[0].ant_name:
                                        surplus.remove(w)
                                        if w.wait_value > cw[0].wait_value:
                                            cand.sync_info = mybir.SyncInfo(
                                                on_wait=[w],
                                                on_update=list(
                                                    csi.on_update))
                            j -= 1
                        assert not surplus, (
                            f"{inst.name} ({type(inst).__name__}, {eng}): "
                            f"could not place surplus waits "
                            + str([w.ant_name for w in si.on_wait]))
                    inst.sync_info = mybir.SyncInfo(
                        on_wait=waits, on_update=list(si.on_update))


def _fix_tail_drain_waits(nc, mybir):
    """The kernel-tail SP drain carries one wait per engine + per DMA
    queue (11 total) - over the CTRL struct's sync-wait budget. The
    engine waits are redundant: the all-engine barrier that follows
    makes every engine drain itself before the final gather. The DMA
    queue waits are spread one-per-instruction over the tail's
    zero-wait instructions (branches / pool drains), all of which
    execute before the NEFF completion signal."""
    insts = [i for fn in nc.m.functions for b in fn.blocks
             for i in b.instructions]
    # locate the big drain
    big = None
    for inst in insts:
        si = inst.sync_info
        if (type(inst).__name__ == "InstDrain" and si is not None
                and len(si.on_wait) > 1):
            assert big is None, "two multi-wait drains?"
            big = inst
    if big is None:
        return
    idx = insts.index(big)
    tail = insts[idx + 1:]
    assert any("barrier" in (t.name or "") for t in tail), \
        "no barrier after the multi-wait drain"
    waits = list(big.sync_info.on_wait)
    dma_waits = [w for w in waits if "DMA" in w.ant_name]
    keep = dma_waits[:1]
    spill = dma_waits[1:]
    if spill:
        # Safe hosts must execute BEFORE the tail's semaphore reset
        # (the Pool InstISA): the per-engine end-of-body branches right
        # before the drain, and the Pool tail instructions up to and
        # including the reset itself (a wait executes before its own
        # instruction's effect).
        hosts = []          # zero-wait instructions
        trivial_hosts = []  # instructions whose single wait is >=0
        for t in insts[max(0, idx - 6):idx]:
            si = t.sync_info
            if (si is None or len(si.on_wait) == 0) and \
                    type(t).__name__ == "InstUnconditionalBranch":
                hosts.append(t)
        for t in tail:
            si = t.sync_info
            tn = type(t).__name__
            if tn == "InstISA":
                break
            if tn not in ("InstEventSemaphore", "InstDrain"):
                continue
            if si is None or len(si.on_wait) == 0:
                hosts.append(t)
        for t in tail:
            si = t.sync_info
            if (type(t).__name__ == "InstDrain" and si is not None
                    and len(si.on_wait) == 1
                    and si.on_wait[0].wait_value == 0):
                trivial_hosts.append(t)
        assert len(hosts) + len(trivial_hosts) >= len(spill), \
            f"not enough pre-reset hosts: {len(hosts)}+" \
            f"{len(trivial_hosts)} < {len(spill)}"
        for w in spill:
            if hosts:
                h = hosts.pop(0)
            else:
                h = trivial_hosts.pop(0)
            hsi = h.sync_info
            h.sync_info = mybir.SyncInfo(
                on_wait=[w],
                on_update=list(hsi.on_update) if hsi else [])
    big.sync_info = mybir.SyncInfo(
        on_wait=keep, on_update=list(big.sync_info.on_update))


_BUILT = {}


def _get_nc(D):
    if D not in _BUILT:
        _BUILT[D] = build_kernel(D)
    return _BUILT[D]


def _install_trace_hook():
    """antenv.axon_hooks is absent in this image; synthesize it so
    run_bass_kernel_spmd(trace=True) can reach the NTFF profiler."""
    import types
    try:
        from antenv.axon_hooks import get_axon_ntff_profile_hook  # noqa
        return True
    except ImportError:
        pass
    try:
        import antenv
        from trn_agent_boot.trn_boot import _ntff_profile_via_ctypes
        hook = _ntff_profile_via_ctypes("/opt/axon/libaxon_pjrt.so")
        mod = types.ModuleType("antenv.axon_hooks")
        mod._hook = hook
        mod.set_axon_ntff_profile_hook = lambda h: setattr(mod, "_hook", h)
        mod.get_axon_ntff_profile_hook = lambda: mod._hook
        sys.modules["antenv.axon_hooks"] = mod
        antenv.axon_hooks = mod
        return True
    except Exception as e:  # pragma: no cover - tracing is best-effort
        print(f"trace hook install failed: {e}", file=sys.stderr)
        return False


def run_cores(in_maps, D, trace=False):
    from concourse.bass_utils import run_bass_kernel_spmd
    if trace:
        trace = _install_trace_hook()
    nc = _get_nc(D)
    res = run_bass_kernel_spmd(nc, in_maps, core_ids=list(range(N_CORES)),
                               trace=trace)
    return res


LAST_EXEC_NS = None


def kernel(x, h0, c0, W_ih, W_hh, b_ih, b_hh, W_out, b_out, D):
    global LAST_EXEC_NS
    D = int(D)
    x = np.asarray(x, np.float32)
    h0 = np.asarray(h0, np.float32)
    c0 = np.asarray(c0, np.float32)

    shared = _prep_shared(np.asarray(W_ih), np.asarray(W_hh),
                          np.asarray(b_ih), np.asarray(b_hh),
                          np.asarray(W_out), np.asarray(b_out))
    B = x.shape[0]
    assert B == N_CORES * B_LOC, f"unexpected batch {B}"
    in_maps = []
    for i in range(N_CORES):
        sl = slice(i * B_LOC, (i + 1) * B_LOC)
        m = dict(shared)
        m.update(_prep_core(x[sl], h0[sl], c0[sl]))
        in_maps.append(m)

    trace = bool(int(os.environ.get("BASS_KERNEL_TRACE", "0")))
    res = run_cores(in_maps, D, trace=trace)
    LAST_EXEC_NS = res.exec_time_ns

    # gather: per-core out is (D, B_LOC, NOUT); full batch on axis 1
    full = np.concatenate([res.results[i]["out"] for i in range(N_CORES)],
                          axis=1)
    return np.ascontiguousarray(full.astype(np.float32))



# revision 18
# speedup vs baseline: 1.0172x; 1.0172x over previous
"""Trainium2 Bass kernel for nn_Actor LSTM scan problem.

Reference computation (per problem):
    xW = x @ W_ih.T + b_ih                       (B, 4H)
    for t in range(D):
        gates = xW + h @ W_hh.T + b_hh           (B, 4H)  order i,f,g,o
        c = sig(f)*c + sig(i)*tanh(g)
        h = sig(o)*tanh(c)
        out[t] = h @ W_out.T + b_out             (B, N)

Shapes: B=512, IN=H=1024, D=64, N=512.

Strategy: pure data parallel over 8 NeuronCores (64 batch rows each, weights
replicated; no collectives). Per core the recurrence runs on the TensorEngine
with the batch as the matmul output-partition dim (M=64), stationary = h.T
K-tiles, moving = W_hh.T. The two free column-group halves of the 128x128 PE
array are used as two concurrent matmul streams (tile_position col split):
group A (psum partitions 0-63) computes gate columns 0-511 of each gate,
group B (psum partitions 64-127) computes gate columns 512-1023. All compute
in bf16 with fp32 PSUM accumulation; c state kept in fp32.

PSUM bank layout per step (bank order): g, i, f, o gates; each bank holds
[128, 512] = A-half rows 0-63 (lo gate cols) / B-half rows 64-127 (hi cols).
xW (+ biases) is pre-computed once on-device and re-injected into PSUM every
step via a single identity-stationary matmul per bank (start=True).
"""

import os
import sys

for _p in ("/opt/trn_rl_repo", "/root/.axon_site/_ro/trn_rl_repo"):
    if os.path.isdir(_p) and _p not in sys.path:
        sys.path.insert(0, _p)

import numpy as np
import ml_dtypes

BF = ml_dtypes.bfloat16

N_CORES = 8
B_LOC = 64    # batch rows per core
H = 1024
IN = 1024
NOUT = 512
KT = 8        # K tiles of 128 along the hidden/contraction dim

# gate blocks in torch order within the 4H dim: i, f, g, o
_GI, _GF, _GG, _GO = 0, 1024, 2048, 3072
# bank order: g, i, f, o ; A = cols 0:512 of each gate, B = cols 512:1024
COLS_A = np.concatenate([
    np.arange(_GG, _GG + 512),
    np.arange(_GI, _GI + 512),
    np.arange(_GF, _GF + 512),
    np.arange(_GO, _GO + 512),
])
COLS_B = COLS_A + 512


def _tile_k(m):
    """(KT*128, C) -> (128, KT, C) so dma is a straight copy into sbuf."""
    c = m.shape[1]
    return np.ascontiguousarray(
        m.reshape(KT, 128, c).transpose(1, 0, 2))


def _prep_shared(W_ih, W_hh, b_ih, b_hh, W_out, b_out):
    """Host-side layout prep of the replicated weights (numpy, float32 in)."""
    WihT = np.asarray(W_ih, np.float32).T   # (IN, 4H)
    WhhT = np.asarray(W_hh, np.float32).T   # (H, 4H)
    WoT = np.asarray(W_out, np.float32).T   # (H, N)
    b4 = (np.asarray(b_ih, np.float32) + np.asarray(b_hh, np.float32))

    d = {}
    d["wiA"] = _tile_k(WihT[:, COLS_A]).astype(BF)
    d["wiB"] = _tile_k(WihT[:, COLS_B]).astype(BF)
    d["whA"] = _tile_k(WhhT[:, COLS_A]).astype(BF)
    d["whB"] = _tile_k(WhhT[:, COLS_B]).astype(BF)
    d["woA"] = _tile_k(WoT[:, :256]).astype(BF)
    d["woB"] = _tile_k(WoT[:, 256:]).astype(BF)
    d["b42"] = np.stack([b4[COLS_A], b4[COLS_B]]).astype(BF)
    bo = np.asarray(b_out, np.float32)
    d["bo2"] = np.stack([bo[:256], bo[256:]]).astype(BF)
    sel2 = np.zeros((2, 128), np.float32)
    sel2[0, :64] = 1.0
    sel2[1, 64:] = 1.0
    d["sel2"] = sel2.astype(BF)
    idm = np.zeros((128, 128), np.float32)
    idm[:128, :128] = np.eye(128)
    d["idm"] = idm.astype(BF)
    return d


def _prep_core(x_sh, h0_sh, c0_sh):
    """Per-core input layout prep."""
    d = {}
    d["xaT"] = _tile_k(np.asarray(x_sh, np.float32).T).astype(BF)
    d["h0T"] = _tile_k(np.asarray(h0_sh, np.float32).T).astype(BF)
    c0 = np.asarray(c0_sh, np.float32)
    d["c0s"] = np.concatenate([c0[:, :512], c0[:, 512:]], axis=0)
    return d


def build_kernel(D):
    import concourse.bass as bass
    import concourse.tile as tile
    from concourse import bacc, mybir
    from contextlib import ExitStack

    f32 = mybir.dt.float32
    bf16 = mybir.dt.bfloat16
    Sig = mybir.ActivationFunctionType.Sigmoid
    Tanh = mybir.ActivationFunctionType.Tanh

    nc = bacc.Bacc()

    def din(name, shape, dt):
        return nc.declare_dram_parameter(name, list(shape), dt, isOutput=False)

    xaT_d = din("xaT", (128, KT, 64), bf16)
    h0T_d = din("h0T", (128, KT, 64), bf16)
    c0s_d = din("c0s", (128, 512), f32)
    wiA_d = din("wiA", (128, KT, 2048), bf16)
    wiB_d = din("wiB", (128, KT, 2048), bf16)
    whA_d = din("whA", (128, KT, 2048), bf16)
    whB_d = din("whB", (128, KT, 2048), bf16)
    woA_d = din("woA", (128, KT, 256), bf16)
    woB_d = din("woB", (128, KT, 256), bf16)
    b42_d = din("b42", (2, 2048), bf16)
    bo2_d = din("bo2", (2, 256), bf16)
    sel2_d = din("sel2", (2, 128), bf16)
    idm_d = din("idm", (128, 128), bf16)
    out_d = nc.declare_dram_parameter("out", [D, B_LOC, NOUT], f32, isOutput=True)

    with tile.TileContext(nc) as tc, ExitStack() as ctx:
        wp = ctx.enter_context(tc.tile_pool(name="wp", bufs=1))
        sp = ctx.enter_context(tc.tile_pool(name="sp", bufs=2))
        op = ctx.enter_context(tc.tile_pool(name="op", bufs=3))
        pp = ctx.enter_context(
            tc.tile_pool(name="pp", bufs=1, space=bass.MemorySpace.PSUM))

        # ---- constants / weights into SBUF ----
        idm = wp.tile([128, 128], bf16, tag="idm")
        nc.sync.dma_start(idm[:], idm_d[:])
        sel2 = wp.tile([2, 128], bf16, tag="sel2")
        nc.sync.dma_start(sel2[:], sel2_d[:])

        xa = wp.tile([128, KT, 64], bf16, tag="xa")
        nc.sync.dma_start(xa[:], xaT_d[:])
        wi = {}
        for g, dd in (("A", wiA_d), ("B", wiB_d)):
            wi[g] = wp.tile([128, KT, 2048], bf16, tag=f"wi{g}", name=f"wi{g}")
            nc.sync.dma_start(wi[g][:], dd[:])
        b42 = wp.tile([2, 2048], bf16, tag="b42")
        nc.sync.dma_start(b42[:], b42_d[:])
        wh = {}
        for g, dd in (("A", whA_d), ("B", whB_d)):
            wh[g] = wp.tile([128, KT, 2048], bf16, tag=f"wh{g}", name=f"wh{g}")
            nc.sync.dma_start(wh[g][:], dd[:])
        wo = {}
        for g, dd in (("A", woA_d), ("B", woB_d)):
            wo[g] = wp.tile([128, KT, 256], bf16, tag=f"wo{g}", name=f"wo{g}")
            nc.sync.dma_start(wo[g][:], dd[:])
        bo2 = wp.tile([2, 256], bf16, tag="bo2")
        nc.sync.dma_start(bo2[:], bo2_d[:])

        # initial state: hT k-tiles 0-3 in hta, 4-7 in htb
        hta = sp.tile([128, 4, 64], bf16, tag="hta", bufs=2, name="hta_init")
        nc.sync.dma_start(hta[:], h0T_d[:, 0:4, :])
        htb = sp.tile([128, 4, 64], bf16, tag="htb", bufs=2, name="htb_init")
        nc.sync.dma_start(htb[:], h0T_d[:, 4:8, :])
        c_cur = sp.tile([128, 512], f32, tag="c", bufs=2)
        nc.sync.dma_start(c_cur[:], c0s_d[:])

        PO = {"A": 0, "B": 64}  # partition offset per group

        def ht_sl(ha, hb, k):
            return ha[:, k, :] if k < 4 else hb[:, k - 4, :]

        # ---- xW precompute: xw[p, 2048] bf16 (A half rows 0-63, B rows 64-127)
        xw = wp.tile([128, 2048], bf16, tag="xw")
        for b in range(4):
            sl = slice(512 * b, 512 * b + 512)
            pt = pp.tile([128, 512], f32, tag=f"pb{b}",
                         bufs=2 if b == 3 else 1,
                         name=f"xwpre{b}")
            nc.tensor.matmul(pt[:, :], sel2[:, :], b42[:, sl],
                             start=True, stop=False,
                             skip_group_check=True)
            for k in range(KT):
                for g in ("A", "B"):
                    po = PO[g]
                    hsl = slice(po, po + 64)
                    nc.tensor.matmul(pt[hsl, :], xa[:, k, :],
                                     wi[g][:, k, sl],
                                     start=False,
                                     stop=(k == KT - 1),
                                     skip_group_check=True)
            nc.vector.tensor_copy(xw[:, sl], pt[:, :])

        # ---- helpers ----
        def emit_outproj(t, ha, hb):
            pout = pp.tile([128, 256], f32, tag="pout", bufs=1,
                           name=f"pout{t}")
            nc.tensor.matmul(pout[:, :], sel2[:, :], bo2[:, :],
                             start=True, stop=False,
                             skip_group_check=True)
            for k in range(KT):
                for g in ("A", "B"):
                    po = PO[g]
                    hsl = slice(po, po + 64)
                    nc.tensor.matmul(pout[hsl, :], ht_sl(ha, hb, k),
                                     wo[g][:, k, :],
                                     start=False, stop=(k == KT - 1),
                                     skip_group_check=True)
            ost = op.tile([128, 256], f32, tag="ost", bufs=4,
                          name=f"ost{t}")
            nc.vector.tensor_copy(ost[:, :], pout[:, :])
            dst = out_d[t].rearrange("b (c n) -> c b n", c=2)
            nc.sync.dma_start(dst, ost[:, :])

        # ---- the scan ----
        for t in range(D):
            # gates matmuls (hta/htb hold hT(h_{t-1}))
            pts = []
            for b in range(4):
                sl = slice(512 * b, 512 * b + 512)
                # pb3 (o gate) is double-buffered: its ACT read (sig o) is the
                # last epilogue op, and bufs=2 lets next step's xW inject run
                # without waiting on it.
                pt = pp.tile([128, 512], f32, tag=f"pb{b}",
                             bufs=2 if b == 3 else 1,
                             name=f"pt{t}_{b}")
                if b == 3:
                    # o bank runs as two independent 256-col accumulation
                    # groups: sig(o) of the first half fires while the second
                    # half's matmuls still stream, pulling h forward.
                    for oh in range(2):
                        osl = slice(256 * oh, 256 * oh + 256)
                        wsl = slice(512 * b + 256 * oh, 512 * b + 256 * oh + 256)
                        nc.tensor.matmul(pt[:, osl], idm[:, :], xw[:, wsl],
                                         start=True, stop=False,
                                         skip_group_check=True)
                        for k in range(KT):
                            for g in ("A", "B"):
                                po = PO[g]
                                hsl = slice(po, po + 64)
                                nc.tensor.matmul(pt[hsl, osl],
                                                 ht_sl(hta, htb, k),
                                                 wh[g][:, k, wsl],
                                                 start=False,
                                                 stop=(k == KT - 1),
                                                 skip_group_check=True)
                else:
                    # inject xW (+biases) into the whole bank in one
                    # full-array MM
                    nc.tensor.matmul(pt[:, :], idm[:, :], xw[:, sl],
                                     start=True, stop=False,
                                     skip_group_check=True)
                    for k in range(KT):
                        for g in ("A", "B"):
                            po = PO[g]
                            hsl = slice(po, po + 64)
                            nc.tensor.matmul(pt[hsl, :], ht_sl(hta, htb, k),
                                             wh[g][:, k, sl],
                                             start=False, stop=(k == KT - 1),
                                             skip_group_check=True)
                pts.append(pt)

            # fill TE while epilogue runs: output projection of previous step
            if t > 0:
                emit_outproj(t - 1, hta, htb)

            # epilogue: c = sig(f)*c + sig(i)*tanh(g); h = sig(o)*tanh(c)
            # banks: 0=g, 1=i, 2=f, 3=o.  f/o + the c->h chain are split in
            # 256-col halves so the tail latency pipelines; so/tcc are bf16
            # to get 2x DVE mode on the final h multiply.
            HL = [slice(0, 256), slice(256, 512)]
            tg = op.tile([128, 512], f32, tag="tg", bufs=2, name=f"tg{t}")
            nc.scalar.activation(tg[:, :], pts[0][:, :], Tanh)
            si = op.tile([128, 512], f32, tag="si", bufs=2, name=f"si{t}")
            nc.scalar.activation(si[:, :], pts[1][:, :], Sig)
            sf = op.tile([128, 512], f32, tag="sf", bufs=2, name=f"sf{t}")
            nc.scalar.activation(sf[:, 0:256], pts[2][:, 0:256], Sig)
            nc.scalar.activation(sf[:, 256:512], pts[2][:, 256:512], Sig)

            t2 = op.tile([128, 512], f32, tag="t2", bufs=2, name=f"t2_{t}")
            nc.vector.tensor_mul(t2[:, :], si[:, :], tg[:, :])

            t1 = op.tile([128, 512], f32, tag="t1", bufs=2, name=f"t1_{t}")
            c_new = sp.tile([128, 512], f32, tag="c", bufs=2, name=f"c{t}")
            tcc = op.tile([128, 512], bf16, tag="tcc", bufs=2, name=f"tcc{t}")
            so = op.tile([128, 512], bf16, tag="so", bufs=2, name=f"so{t}")
            hh = op.tile([128, 512], bf16, tag="hh", bufs=2, name=f"hh{t}")
            for h_ in range(2):
                hsl = HL[h_]
                nc.vector.tensor_mul(t1[:, hsl], sf[:, hsl], c_cur[:, hsl])
                nc.vector.tensor_add(c_new[:, hsl], t1[:, hsl], t2[:, hsl])
            # ACT tail in input-ready order: tcc1, so1 (at o-half0 stop),
            # tcc2, so2; hh halves land right at the o bank boundary.
            # The scheduler's cost model underestimates the DVE chain and
            # would slot tcc2 ahead of so1 in the ACT queue, pushing h back
            # ~0.5us; a zero-valued bias tile derived from so1's output
            # forces tcc2 to sequence after so1.
            nc.scalar.activation(tcc[:, 0:256], c_new[:, 0:256], Tanh)
            nc.scalar.activation(so[:, 0:256], pts[3][:, 0:256], Sig)
            zb = op.tile([128, 1], f32, tag="zb", bufs=2, name=f"zb{t}")
            nc.vector.tensor_scalar_mul(zb[:, :], so[:, 0:1], 0.0)
            nc.scalar.activation(tcc[:, 256:512], c_new[:, 256:512], Tanh,
                                 bias=zb[:, :])
            nc.scalar.activation(so[:, 256:512], pts[3][:, 256:512], Sig)
            for h_ in range(2):
                hsl = HL[h_]
                nc.vector.tensor_mul(hh[:, hsl], so[:, hsl], tcc[:, hsl])
            c_cur = c_new

            # transpose h -> hT tiles: regular matmuls with identity moving
            # operand; the upper batch half reads partitions 64-127 directly
            # (row tile position 64), no DRAM roundtrip.
            pa = pp.tile([128, 256], f32, tag="pa", bufs=1, name=f"pa{t}")
            pb = pp.tile([128, 256], f32, tag="pb_t", bufs=1, name=f"pbt{t}")
            for j in range(4):
                nc.tensor.matmul(pa[:, 64 * j:64 * j + 64],
                                 hh[0:64, 128 * j:128 * j + 128],
                                 idm[0:64, 0:64],
                                 start=True, stop=True,
                                 skip_group_check=True)
            for j in range(4):
                nc.tensor.matmul(pb[:, 64 * j:64 * j + 64],
                                 hh[64:128, 128 * j:128 * j + 128],
                                 idm[64:128, 64:128],
                                 start=True, stop=True,
                                 skip_group_check=True)
            hta = sp.tile([128, 4, 64], bf16, tag="hta", bufs=2,
                          name=f"hta{t}")
            htb = sp.tile([128, 4, 64], bf16, tag="htb", bufs=2,
                          name=f"htb{t}")
            nc.vector.tensor_copy(
                hta[:, 0:2, :], pa[:, 0:128].rearrange("p (k n) -> p k n", k=2))
            nc.vector.tensor_copy(
                hta[:, 2:4, :], pa[:, 128:256].rearrange("p (k n) -> p k n", k=2))
            nc.vector.tensor_copy(
                htb[:, 0:2, :], pb[:, 0:128].rearrange("p (k n) -> p k n", k=2))
            nc.vector.tensor_copy(
                htb[:, 2:4, :], pb[:, 128:256].rearrange("p (k n) -> p k n", k=2))

        emit_outproj(D - 1, hta, htb)

    if not nc.is_finalized():
        nc.finalize()
    return nc


def _fix_matmul_waits(nc, mybir):
    """Walrus allows only one sync-wait slot per compute instruction
    (PE matmul / ldweights, DVE, ACT, DMA trigger). Tile sometimes
    emits 2-3. Legalize each engine stream:
      1. drop waits on the instruction's own update semaphore (engine
         streams and DMA rings complete in order - guaranteed by
         program order);
      2. hoist surplus waits onto the nearest preceding same-engine
         instruction with a free wait slot (conservative: a wait moved
         earlier on the same serial stream still gates the original
         instruction; the producers of such waits are always at least
         one pool-buf generation older, so no cycles arise within the
         short hoist window).
    """
    LIMIT1 = {"InstMatmult", "InstLdweights", "InstTensorTensor",
              "InstTensorCopy", "InstActivation", "InstTensorScalarPtr",
              "InstMemset", "InstTensorReduce", "InstCopyPredicated",
              "InstReciprocal", "InstDMACopy"}
    for fn in nc.m.functions:
        for blk in fn.blocks:
            insts = [i for i in blk.instructions if type(i).__name__
                     in LIMIT1]
            by_eng = {}
            for inst in insts:
                by_eng.setdefault(str(inst.engine), []).append(inst)
            for eng, stream in by_eng.items():
                for pos, inst in enumerate(stream):
                    si = inst.sync_info
                    if si is None or len(si.on_wait) <= 1:
                        continue
                    own = {u.ant_name for u in si.on_update}
                    if type(inst).__name__ == "InstDMACopy" and \
                            os.environ.get("KFIX_DMA_KEEP_RING", "0") == "1":
                        waits = list(si.on_wait)
                    else:
                        waits = [w for w in si.on_wait
                                 if w.ant_name not in own]
                    if len(waits) > 1:
                        # keep a cross-engine compute (RAW) wait on the
                        # instruction; hoist the DMA (pool-reuse WAR)
                        # waits onto nearby predecessors
                        if type(inst).__name__ == "InstDMACopy":
                            keep = next(
                                (w for w in waits if w.ant_name in own),
                                waits[-1])
                        else:
                            keep = next(
                                (w for w in waits
                                 if "DMA" not in w.ant_name),
                                waits[-1])
                        surplus = [w for w in waits if w is not keep]
                        waits = [keep]
                        j = pos - 1
                        while surplus and j >= 0 and j >= pos - 24:
                            cand = stream[j]
                            csi = cand.sync_info
                            cw = list(csi.on_wait) if csi else []
                            if len(cw) == 0:
                                w = surplus.pop()
                                cand.sync_info = mybir.SyncInfo(
                                    on_wait=[w],
                                    on_update=list(csi.on_update)
                                    if csi else [])
                            elif len(cw) == 1:
                                # merge if same semaphore
                                for w in list(surplus):
                                    if w.ant_name == cw[0].ant_name:
                                        surplus.remove(w)
                                        if w.wait_value > cw[0].wait_value:
                                            cand.sync_info = mybir.SyncInfo(
                                                on_wait=[w],
                                                on_update=list(
                                                    csi.on_update))
                            j -= 1
                        assert not surplus, (
                            f"{inst.name} ({type(inst).__name__}, {eng}): "
                            f"could not place surplus waits "
                            + str([w.ant_name for w in si.on_wait]))
                    inst.sync_info = mybir.SyncInfo(
                        on_wait=waits, on_update=list(si.on_update))


def _fix_tail_drain_waits(nc, mybir):
    """The kernel-tail SP drain carries one wait per engine + per DMA
    queue (11 total) - over the CTRL struct's sync-wait budget. The
    engine waits are redundant: the all-engine barrier that follows
    makes every engine drain itself before the final gather. The DMA
    queue waits are spread one-per-instruction over the tail's
    zero-wait instructions (branches / pool drains), all of which
    execute before the NEFF completion signal."""
    insts = [i for fn in nc.m.functions for b in fn.blocks
             for i in b.instructions]
    # locate the big drain
    big = None
    for inst in insts:
        si = inst.sync_info
        if (type(inst).__name__ == "InstDrain" and si is not None
                and len(si.on_wait) > 1):
            assert big is None, "two multi-wait drains?"
            big = inst
    if big is None:
        return
    idx = insts.index(big)
    tail = insts[idx + 1:]
    assert any("barrier" in (t.name or "") for t in tail), \
        "no barrier after the multi-wait drain"
    waits = list(big.sync_info.on_wait)
    dma_waits = [w for w in waits if "DMA" in w.ant_name]
    keep = dma_waits[:1]
    spill = dma_waits[1:]
    if spill:
        # Safe hosts must execute BEFORE the tail's semaphore reset
        # (the Pool InstISA): the per-engine end-of-body branches right
        # before the drain, and the Pool tail instructions up to and
        # including the reset itself (a wait executes before its own
        # instruction's effect).
        hosts = []          # zero-wait instructions
        trivial_hosts = []  # instructions whose single wait is >=0
        for t in insts[max(0, idx - 6):idx]:
            si = t.sync_info
            if (si is None or len(si.on_wait) == 0) and \
                    type(t).__name__ == "InstUnconditionalBranch":
                hosts.append(t)
        for t in tail:
            si = t.sync_info
            tn = type(t).__name__
            if tn == "InstISA":
                break
            if tn not in ("InstEventSemaphore", "InstDrain"):
                continue
            if si is None or len(si.on_wait) == 0:
                hosts.append(t)
        for t in tail:
            si = t.sync_info
            if (type(t).__name__ == "InstDrain" and si is not None
                    and len(si.on_wait) == 1
                    and si.on_wait[0].wait_value == 0):
                trivial_hosts.append(t)
        assert len(hosts) + len(trivial_hosts) >= len(spill), \
            f"not enough pre-reset hosts: {len(hosts)}+" \
            f"{len(trivial_hosts)} < {len(spill)}"
        for w in spill:
            if hosts:
                h = hosts.pop(0)
            else:
                h = trivial_hosts.pop(0)
            hsi = h.sync_info
            h.sync_info = mybir.SyncInfo(
                on_wait=[w],
                on_update=list(hsi.on_update) if hsi else [])
    big.sync_info = mybir.SyncInfo(
        on_wait=keep, on_update=list(big.sync_info.on_update))


_BUILT = {}


def _get_nc(D):
    if D not in _BUILT:
        _BUILT[D] = build_kernel(D)
    return _BUILT[D]


def _install_trace_hook():
    """antenv.axon_hooks is absent in this image; synthesize it so
    run_bass_kernel_spmd(trace=True) can reach the NTFF profiler."""
    import types
    try:
        from antenv.axon_hooks import get_axon_ntff_profile_hook  # noqa
        return True
    except ImportError:
        pass
    try:
        import antenv
        from trn_agent_boot.trn_boot import _ntff_profile_via_ctypes
        hook = _ntff_profile_via_ctypes("/opt/axon/libaxon_pjrt.so")
        mod = types.ModuleType("antenv.axon_hooks")
        mod._hook = hook
        mod.set_axon_ntff_profile_hook = lambda h: setattr(mod, "_hook", h)
        mod.get_axon_ntff_profile_hook = lambda: mod._hook
        sys.modules["antenv.axon_hooks"] = mod
        antenv.axon_hooks = mod
        return True
    except Exception as e:  # pragma: no cover - tracing is best-effort
        print(f"trace hook install failed: {e}", file=sys.stderr)
        return False


def run_cores(in_maps, D, trace=False):
    from concourse.bass_utils import run_bass_kernel_spmd
    if trace:
        trace = _install_trace_hook()
    nc = _get_nc(D)
    res = run_bass_kernel_spmd(nc, in_maps, core_ids=list(range(N_CORES)),
                               trace=trace)
    return res


LAST_EXEC_NS = None


def kernel(x, h0, c0, W_ih, W_hh, b_ih, b_hh, W_out, b_out, D):
    global LAST_EXEC_NS
    D = int(D)
    x = np.asarray(x, np.float32)
    h0 = np.asarray(h0, np.float32)
    c0 = np.asarray(c0, np.float32)

    shared = _prep_shared(np.asarray(W_ih), np.asarray(W_hh),
                          np.asarray(b_ih), np.asarray(b_hh),
                          np.asarray(W_out), np.asarray(b_out))
    B = x.shape[0]
    assert B == N_CORES * B_LOC, f"unexpected batch {B}"
    in_maps = []
    for i in range(N_CORES):
        sl = slice(i * B_LOC, (i + 1) * B_LOC)
        m = dict(shared)
        m.update(_prep_core(x[sl], h0[sl], c0[sl]))
        in_maps.append(m)

    trace = bool(int(os.environ.get("BASS_KERNEL_TRACE", "0")))
    res = run_cores(in_maps, D, trace=trace)
    LAST_EXEC_NS = res.exec_time_ns

    # gather: per-core out is (D, B_LOC, NOUT); full batch on axis 1
    full = np.concatenate([res.results[i]["out"] for i in range(N_CORES)],
                          axis=1)
    return np.ascontiguousarray(full.astype(np.float32))

